# revision 12
# baseline (speedup 1.0000x reference)
"""Banded multi-headed attention (nn_BandedMultiheadedAttention) on 8 Trainium2 NeuronCores.

Sharding: data-parallel over (batch, sequence-chunk): core c handles batch c//4,
query positions [256*(c%4), 256*(c%4)+256). Band halo (max (KC-1)*dil = 248) is
loaded per-core with zero padding (projection of zero rows reproduces the
reference's bias padding exactly).

All-fp16 pipeline (fp32 PSUM accumulation, fp32 softmax). Deeply pipelined
schedule: per-subhead Q/K projections feed per-(chunk, subhead-group) score
planes; the banded shear gather/scatter DRAM round trips are split across the
two HWDGE rings (chunk 0 -> sync, chunk 1 -> scalar) with per-group DRAM
tensors so dependency tracking stays fine-grained; V projection fills the PE
during the softmax/scatter windows; per-head-group W planes let the PV phase
start as soon as the dil=1 readback lands. The dil=1 W planes are per-chunk
compact strips (160 cols/head), halving their zero-fill and readback.

Key invariant (HW-verified): multi-dim DMA APs pair source/dest element
streams lexicographically, so the dil>1 band gather permutes band rows by
tau_dil (p = rho*(128/dil)+qq -> q = dil*qq+rho) and the W scatter applies the
inverse; softmax is row-local per head, so the permutations cancel. The
gather/scatter AP dimension structure below must not be changed.
"""

import contextlib
import ctypes
import sys
import types

import numpy as np

# ---------------------------------------------------------------- constants
B, N, D = 2, 1024, 640
DH, KC, SUBHEADS, HEADS = 128, 32, 5, 14
Q = 256                      # query positions per core
NCORES = 8
HALO = 124                   # (KC-1)*max_dil // 2
KV = 512                     # kv halo positions per core ([t0-124, t0+388))
KVX = 640                    # zero-extended vT columns (deint APs over-reach)

DIL_S = [1, 1, 2, 4, 8]
SUPER = [5, 5, 2, 1, 1]
DIL_H = [1] * 10 + [2, 2, 4, 8]
PL_S = [(KC - 1) * d // 2 for d in DIL_S]
OFF_S = [HALO - p for p in PL_S]                   # kT col of m=0 per subhead
PL_H = [(KC - 1) * d // 2 for d in DIL_H]
OFF_H = [HALO - p for p in PL_H]

M_S = [288, 288, 320, 384, 512]                    # scores width per subhead

SPL01_W = 2 * M_S[0]                               # s0|s1 score plane (576)
SOFF234 = [0, M_S[2], M_S[2] + M_S[3]]             # s2,s3,s4 within spl234
SPL234_W = M_S[2] + M_S[3] + M_S[4]                # 1216

# W planes. A group (heads 0-9, dil=1): per-chunk compact strips of SA=160
# cols (local m = q_local + j); chunk c's strip maps to global positions
# [c*128, c*128+160) so vtile chunk (c + local_chunk) rows align 1:1.
# B group (heads 10-13): global deint planes, md = M/dil multiple of 64.
SA = 160
WLA = 10 * SA + 96                                 # A plane width (+96 pad so
                                                   # chunk-1 transposes read 128 cols)
M_H = [384] * 10 + [512] * 4
WLDB = 4 * 512                                     # heads 10-13 plane width
WOFFB = [0, 512, 1024, 1536]
MC_H = [m // 128 for m in M_H]

# V-projection head packs (same dilation within a pack)
PACKS = [[0, 1, 2, 3], [4, 5, 6, 7], [8, 9], [10, 11], [12], [13]]
PACK_OF_H = {h: (p, g.index(h)) for p, g in enumerate(PACKS) for h in g}
PACK_OFF = [OFF_H[g[0]] for g in PACKS]
PACK_MC = [MC_H[g[0]] for g in PACKS]

HJ = HEADS * KC  # 448


def _chunk_rows(h_or_p, mc, head=True):
    """Row (t, rho) segments of 128-col W-plane chunk mc: returns list of
    (row_in_chunk, rho, t0, seg_len); positions are p = OFF + dil*t + rho."""
    dil = DIL_H[h_or_p] if head else DIL_H[PACKS[h_or_p][0]]
    M = M_H[h_or_p] if head else M_H[PACKS[h_or_p][0]]
    md = M // dil
    segs = []
    r = 0
    while r < 128:
        col0 = mc * 128 + r
        rho, t0 = col0 // md, col0 % md
        seg = min(128 - r, md - t0)
        segs.append((r, rho, t0, seg))
        r += seg
    return segs


def _live_cs(h, mc):
    """Which q-halves c have any band data in W-plane chunk (h, mc)."""
    dil = DIL_H[h]
    lives = []
    for c in range(2):
        lo, hi = c * 128, c * 128 + 127 + (KC - 1) * dil
        ok = False
        for _, rho, t0, seg in _chunk_rows(h, mc):
            p0, p1 = dil * t0 + rho, dil * (t0 + seg - 1) + rho
            if p0 <= hi and p1 >= lo:
                ok = True
        if ok:
            lives.append(c)
    return lives


_BUILT = None


def _inject_ntff_hook():
    """bass_utils reads antenv.axon_hooks for NTFF profiling; the module is
    absent in this image. Recreate the ctypes glue (mirrors trn_boot.py)."""
    try:
        import antenv.axon_hooks  # noqa: F401
        return
    except ImportError:
        pass

    def _make(so_path):
        try:
            lib = ctypes.CDLL(so_path)
        except OSError:
            return None
        if not hasattr(lib, "axon_start_nrt_profile"):
            return None
        lib.axon_start_nrt_profile.argtypes = [ctypes.POINTER(ctypes.c_int64), ctypes.c_size_t]
        lib.axon_start_nrt_profile.restype = ctypes.c_int64
        lib.axon_stop_nrt_profile.argtypes = [ctypes.c_char_p]
        lib.axon_stop_nrt_profile.restype = ctypes.c_int64

        @contextlib.contextmanager
        def _hook(output_dir, device_ids):
            import jax
            jax.devices()
            if device_ids:
                ids = (ctypes.c_int64 * len(device_ids))(*device_ids)
                rc = lib.axon_start_nrt_profile(ids, len(device_ids))
            else:
                rc = lib.axon_start_nrt_profile(None, 0)
            if rc != 0:
                raise RuntimeError(f"axon_start_nrt_profile rc={rc}")
            try:
                yield
            finally:
                n = lib.axon_stop_nrt_profile(str(output_dir).encode())
                print(f"ntff profile: {n} file(s) -> {output_dir}", file=sys.stderr)

        return _hook

    hook = _make("/opt/axon/libaxon_pjrt.so")
    mod = types.ModuleType("antenv.axon_hooks")
    mod.get_axon_ntff_profile_hook = lambda: hook
    mod.set_axon_ntff_profile_hook = lambda h: None
    sys.modules["antenv.axon_hooks"] = mod


def _build():
    """Build the (single) SPMD Bass program. Returns finalized nc."""
    import concourse.bass as bass
    import concourse.tile as tile
    from concourse import bacc, mybir
    from concourse.masks import make_identity
    from concourse.tile import add_dep_helper

    f32 = mybir.dt.float32
    f16 = mybir.dt.float16
    AP = bass.AP

    nc = bacc.Bacc("TRN2", target_bir_lowering=False, debug=False, num_devices=NCORES)

    # ---------------- external IO (fp16 except fp32 bias3 / output)
    qT_d = nc.dram_tensor("qT", [DH, SUBHEADS * Q], f16, kind="ExternalInput")
    kT_d = nc.dram_tensor("kT", [DH, SUBHEADS * KV], f16, kind="ExternalInput")
    vT_d = nc.dram_tensor("vT", [DH, SUBHEADS * KV], f16, kind="ExternalInput")
    QkT_d = nc.dram_tensor("QkT", [DH, SUBHEADS * SUBHEADS * DH], f16, kind="ExternalInput")
    KkT_d = nc.dram_tensor("KkT", [DH, SUBHEADS * SUBHEADS * DH], f16, kind="ExternalInput")
    VG = [512, 512, 512, 256]
    VGP = [[0], [1], [2, 3], [4, 5]]  # packs per group
    Vg_d = [nc.dram_tensor(f"Vg{i}", [DH, SUBHEADS * w], f16, kind="ExternalInput")
            for i, w in enumerate(VG)]
    SkT_d = nc.dram_tensor("SkT", [KC, HJ], f16, kind="ExternalInput")
    SbH_d = nc.dram_tensor("SbH", [1, HJ], f16, kind="ExternalInput")
    CbH_d = nc.dram_tensor("CbH", [1, D], f16, kind="ExternalInput")
    bias3_d = nc.dram_tensor("bias3", [DH, 2 * SUBHEADS + HEADS], f32,
                             kind="ExternalInput")
    CkT_d = nc.dram_tensor("CkT", [DH, HEADS * D], f16, kind="ExternalInput")
    out_d = nc.dram_tensor("out", [Q, D], f32, kind="ExternalOutput")

    # ---------------- internal DRAM scratch, split per chunk and group so the
    # tile framework's DRAM dependency tracking stays fine-grained.
    spl01 = [nc.dram_tensor(f"spl01_{c}", [128, SPL01_W], f16, kind="Internal")
             for c in range(2)]
    spl234 = [nc.dram_tensor(f"spl234_{c}", [128, SPL234_W], f16, kind="Internal")
              for c in range(2)]
    wplA = [nc.dram_tensor(f"wplA{c}", [128, WLA], f16, kind="Internal")
            for c in range(2)]
    wplB = [nc.dram_tensor(f"wplB{c}", [128, WLDB], f16, kind="Internal")
            for c in range(2)]

    with tile.TileContext(nc) as tc, contextlib.ExitStack() as ctx:
        consts = ctx.enter_context(tc.tile_pool(name="consts", bufs=1))
        acts = ctx.enter_context(tc.tile_pool(name="acts", bufs=1))
        work = ctx.enter_context(tc.tile_pool(name="work", bufs=4))
        wftp = ctx.enter_context(tc.tile_pool(name="wft", bufs=6))
        actp = ctx.enter_context(tc.tile_pool(name="actp", bufs=2))
        ps_mm = ctx.enter_context(tc.tile_pool(name="ps_mm", bufs=2, space="PSUM"))
        ps_sm = ctx.enter_context(tc.tile_pool(name="ps_sm", bufs=2, space="PSUM"))
        ps_at = ctx.enter_context(tc.tile_pool(name="ps_at", bufs=2, space="PSUM"))
        ps_co = ctx.enter_context(tc.tile_pool(name="ps_co", bufs=2, space="PSUM"))

        eng2 = [nc.sync, nc.scalar]

        # ---------------- critical inputs: sync carries qT+KkT, scalar
        # carries QkT+kT, so Q-side and K-side complete in parallel.
        qTr = acts.tile([DH, SUBHEADS, Q], f16)
        kTr = acts.tile([DH, SUBHEADS, KV], f16)
        QkTr = consts.tile([DH, SUBHEADS * SUBHEADS, DH], f16)
        KkTr = consts.tile([DH, SUBHEADS * SUBHEADS, DH], f16)

        def pieces(eng, dst, src_d, width, npc):
            ds = []
            for i in range(npc):
                a, b = width * i // npc, width * (i + 1) // npc
                ds.append(eng.dma_start(
                    out=dst[:, a:b], in_=AP(src_d, a, [[width, DH], [1, b - a]])))
            return ds

        qTrf = qTr.rearrange("p a b -> p (a b)")
        kTrf = kTr.rearrange("p a b -> p (a b)")
        QkTrf = QkTr.rearrange("p a b -> p (a b)")
        KkTrf = KkTr.rearrange("p a b -> p (a b)")
        g1 = pieces(nc.sync, qTrf, qT_d, SUBHEADS * Q, 2)
        g2 = pieces(nc.scalar, QkTrf, QkT_d, SUBHEADS * SUBHEADS * DH, 2)
        g3 = pieces(nc.sync, KkTrf, KkT_d, SUBHEADS * SUBHEADS * DH, 2)
        g4 = pieces(nc.scalar, kTrf, kT_d, SUBHEADS * KV, 2)
        bias3 = consts.tile([DH, 2 * SUBHEADS + HEADS], f32)
        nc.gpsimd.dma_start(out=bias3, in_=bias3_d.ap())
        SkT = consts.tile([KC, HJ], f16)
        nc.gpsimd.dma_start(out=SkT, in_=SkT_d.ap())
        SbH = consts.tile([1, HJ], f16)
        nc.gpsimd.dma_start(out=SbH, in_=SbH_d.ap())
        CbH = consts.tile([1, D], f16)
        nc.gpsimd.dma_start(out=CbH, in_=CbH_d.ap())
        gates = [g1[-1], g2[-1], g3[-1], g4[-1]]
        QbT = bias3[:, 0:SUBHEADS]
        VbT = bias3[:, 2 * SUBHEADS :]

        # ---------------- bulk loads on SWDGE (Pool engine), deferred past
        # the critical inputs.
        def gated(d):
            for g in gates:
                add_dep_helper(d.ins, g.ins, sync=True,
                               reason="defer bulk DMA until critical inputs loaded")
            return d

        zrow = work.tile([DH, WLA], f16, name="zrow", tag="zr", bufs=1)
        nc.vector.memset(zrow, 0.0)

        def zeroA(c, eng=None):
            gated((eng or nc.gpsimd).dma_start(
                out=AP(wplA[c], 0, [[WLA, 128], [1, WLA]]), in_=zrow))

        def zeroB(c, eng=None):
            gated((eng or nc.gpsimd).dma_start(
                out=AP(wplB[c], 0, [[WLDB, 128], [1, WLDB]]),
                in_=AP(zrow.tensor, zrow.offset,
                       [[WLA, DH], [0, 2], [1, WLDB // 2]])))

        vT = acts.tile([DH, SUBHEADS, KVX], f16)
        nc.vector.memset(vT[:, :, KV:], 0.0)
        Vgt = [consts.tile([DH, SUBHEADS, w], f16, name=f"Vg{i}")
               for i, w in enumerate(VG)]

        def vg_load(i):
            gated(nc.gpsimd.dma_start(out=Vgt[i].rearrange("p a b -> p (a b)"),
                                      in_=Vg_d[i].ap()))

        gated(nc.gpsimd.dma_start(
            out=AP(vT.tensor, vT.offset,
                   [[SUBHEADS * KVX, DH], [KVX, SUBHEADS], [1, KV]]),
            in_=AP(vT_d, 0, [[SUBHEADS * KV, DH], [KV, SUBHEADS], [1, KV]])))
        zeroA(0)
        zeroB(0)
        vg_load(0)
        vg_load(1)
        vg_load(2)
        zeroA(1)
        zeroB(1)
        vg_load(3)
        # per-pack views into the groups
        Vp = []
        for i, ps in enumerate(VGP):
            off = 0
            for p in ps:
                npk = len(PACKS[p]) * DH
                Vp.append(Vgt[i][:, :, off : off + npk])
                off += npk

        # collapse weights: head-aligned quarters on the HWDGE rings, gated
        # past the critical inputs so they don't steal HBM from kT/KkT
        CkT = consts.tile([DH, HEADS, D], f16)
        CkTf = CkT.rearrange("p a b -> p (a b)")
        CKB = [0, 4 * D, 7 * D, 11 * D, HEADS * D]

        def ckt_piece(i):
            a, b = CKB[i], CKB[i + 1]
            gated(nc.gpsimd.dma_start(out=CkTf[:, a:b],
                                      in_=AP(CkT_d, a, [[HEADS * D, DH], [1, b - a]])))

        for _i in range(4):
            ckt_piece(_i)

        ident = consts.tile([DH, DH], f32)
        make_identity(nc, ident)
        identh = consts.tile([DH, DH], f16)
        nc.vector.tensor_copy(identh, ident)
        ones1 = consts.tile([1, DH], f16)
        nc.vector.memset(ones1, 1.0)

        # ---------------- Q/K projections + scores, pipelined per subhead.
        qTs, kTs = [], []
        ssb01 = [work.tile([128, SPL01_W], f16, name=f"ssb01_{c}", tag="s01", bufs=2)
                 for c in range(2)]
        ssb234 = [work.tile([128, SPL234_W], f16, name=f"ssb234_{c}", tag="s234",
                            bufs=2)
                  for c in range(2)]
        bands = {}

        def proj_scores(s):
            pq = ps_mm.tile([DH, Q], f32, name=f"pq{s}", tag="mm")
            for dc in range(SUBHEADS):
                nc.tensor.matmul(pq, QkTr[:, s * SUBHEADS + dc, :], qTr[:, dc, :],
                                 start=(dc == 0), stop=(dc == SUBHEADS - 1))
            tq = acts.tile([DH, Q], f16, name=f"qTs{s}")
            nc.scalar.activation(tq, pq, mybir.ActivationFunctionType.Identity,
                                 bias=QbT[:, s : s + 1], scale=1.0)
            qTs.append(tq)

            ms = M_S[s]
            pk = ps_mm.tile([DH, ms], f32, name=f"pk{s}", tag="mm")
            for dc in range(SUBHEADS):
                nc.tensor.matmul(pk,
                                 KkTr[:, s * SUBHEADS + dc, :],
                                 kTr[:, dc, OFF_S[s] : OFF_S[s] + ms],
                                 start=(dc == 0), stop=(dc == SUBHEADS - 1))
            tk = acts.tile([DH, ms], f16, name=f"kTs{s}")
            nc.vector.tensor_add(
                tk, pk, AP(bias3.tensor,
                           bias3.offset + SUBHEADS + s,
                           [[2 * SUBHEADS + HEADS, DH], [0, ms]]))
            kTs.append(tk)

            dil = DIL_S[s]
            for c in range(2):
                pscore = ps_sm.tile([128, ms], f32, name=f"psc{s}{c}", tag="sm")
                nc.tensor.matmul(pscore, tq[:, c * 128 : c * 128 + 128],
                                 tk, start=True, stop=True)
                if s < 2:
                    dst = ssb01[c][:, s * M_S[0] : s * M_S[0] + ms]
                    psrc = pscore
                else:
                    # deinterleave m -> (m%dil, m//dil) during PSUM->SBUF copy
                    psrc = AP(pscore.tensor, pscore.offset,
                              [[ms, 128], [1, dil], [dil, ms // dil]])
                    dst = AP(ssb234[c].tensor, ssb234[c].offset + SOFF234[s - 2],
                             [[SPL234_W, 128], [ms // dil, dil], [1, ms // dil]])
                if c == 0:
                    nc.vector.tensor_copy(dst, psrc)
                else:
                    nc.scalar.copy(dst, psrc)

        def w01_roundtrip(c):
            eng = eng2[c]
            eng.dma_start(out=AP(spl01[c], 0, [[SPL01_W, 128], [1, SPL01_W]]),
                          in_=ssb01[c])
            band01 = work.tile([128, 2, KC], f16, name=f"band01_{c}", tag="b01",
                               bufs=2)
            eng.dma_start(
                out=band01,
                in_=AP(spl01[c], c * 128,
                       [[SPL01_W + 1, 128], [M_S[0], 2], [1, KC]]))
            bands[(c, 0)] = band01[:, 0, :]
            bands[(c, 1)] = band01[:, 1, :]

        def w234_roundtrip(c):
            eng = eng2[c]
            eng.dma_start(out=AP(spl234[c], 0, [[SPL234_W, 128], [1, SPL234_W]]),
                          in_=ssb234[c])
            for s in range(2, SUBHEADS):
                dil, ms = DIL_S[s], M_S[s]
                band = work.tile([128, KC], f16, name=f"band{c}{s}", tag="band",
                                 bufs=6)
                eng.dma_start(
                    out=band,
                    in_=AP(spl234[c], SOFF234[s - 2] + (c * 128) // dil,
                           [[SPL234_W + ms // dil, dil],
                            [dil * SPL234_W + 1, 128 // dil], [1, KC]]))
                bands[(c, s)] = band

        proj_scores(0)
        proj_scores(1)
        w01_roundtrip(0)
        w01_roundtrip(1)
        proj_scores(2)
        proj_scores(3)
        proj_scores(4)
        w234_roundtrip(0)
        w234_roundtrip(1)

        # ---------------- V projection tiles (fp16): fill the PE while the
        # band/softmax DRAM round trips are in flight.
        vtiles = {}

        def vproj_packs(plist):
            for p in plist:
                g = PACKS[p]
                npk = len(g) * DH
                dil = DIL_H[g[0]]
                for mc in range(PACK_MC[p]):
                    pv = ps_mm.tile([128, npk], f32, name=f"pv{p}{mc}", tag="mm")
                    segs = _chunk_rows(p, mc, head=False)
                    for dc in range(SUBHEADS):
                        base = vT.offset + dc * KVX
                        if len(segs) == 1:
                            _, rho, t0, _ = segs[0]
                            lhsT = AP(vT.tensor, base + PACK_OFF[p] + dil * t0 + rho,
                                      [[SUBHEADS * KVX, DH], [dil, 128]])
                            nc.tensor.matmul(pv, lhsT, Vp[p][:, dc, :],
                                             start=(dc == 0),
                                             stop=(dc == SUBHEADS - 1))
                        else:
                            # md=64: two residue classes -> two partition-
                            # sliced matmuls (matmul APs allow one free dim)
                            for r, rho, t0, seg in segs:
                                lhsT = AP(vT.tensor,
                                          base + PACK_OFF[p] + dil * t0 + rho,
                                          [[SUBHEADS * KVX, DH], [dil, seg]])
                                nc.tensor.matmul(pv[r : r + seg, :], lhsT,
                                                 Vp[p][:, dc, :],
                                                 start=(dc == 0),
                                                 stop=(dc == SUBHEADS - 1),
                                                 skip_group_check=True)
                    t = acts.tile([128, npk], f16, name=f"v{p}_{mc}")
                    if (p + mc) % 2 == 0:
                        nc.vector.tensor_copy(t, pv)
                    else:
                        nc.scalar.copy(t, pv)
                    vtiles[(p, mc)] = t

        # ---------------- band -> Sk -> exp, per (chunk, subhead-group)
        e_t = [work.tile([128, HJ], f32, name=f"e{c}", tag=f"e{c}", bufs=1)
               for c in range(2)]
        HLO = [0]
        for s in range(SUBHEADS - 1):
            HLO.append(HLO[-1] + SUPER[s] * KC)

        def sk_phase(c, slist):
            for s in slist:
                pbt = ps_sm.tile([KC, 128], f16, name="pbt", tag="sm")
                nc.tensor.transpose(pbt, bands[(c, s)], identh)
                bt = work.tile([KC, 128], f16, name="bt", tag="bt", bufs=6)
                if c == 0:
                    nc.scalar.copy(bt, pbt)
                else:
                    nc.vector.tensor_copy(bt, pbt)
                ncols = SUPER[s] * KC
                psk = ps_sm.tile([128, ncols], f32, name="psk", tag="sm")
                nc.tensor.matmul(psk, bt, SkT[:, HLO[s] : HLO[s] + ncols],
                                 start=True, stop=False, skip_group_check=True)
                # fold the Sb bias in as a rank-1 accumulation
                nc.tensor.matmul(psk, ones1, SbH[:, HLO[s] : HLO[s] + ncols],
                                 start=False, stop=True, skip_group_check=True)
                nc.scalar.activation(e_t[c][:, HLO[s] : HLO[s] + ncols], psk,
                                     mybir.ActivationFunctionType.Exp)

        def softmax_scatter(c):
            e = e_t[c]
            z = work.tile([128, HEADS], f32, name="z", tag="z", bufs=4)
            nc.vector.reduce_sum(z, e.rearrange("p (h k) -> p h k", k=KC),
                                 axis=mybir.AxisListType.X)
            rz = work.tile([128, HEADS], f32, name="rz", tag="z", bufs=4)
            nc.vector.reciprocal(rz, z)
            w = work.tile([128, HJ], f16, name="w", tag="w", bufs=2)
            nc.vector.tensor_mul(
                w.rearrange("p (h k) -> p h k", k=KC),
                e.rearrange("p (h k) -> p h k", k=KC),
                AP(rz.tensor, rz.offset, [[HEADS, 128], [1, HEADS], [0, KC]]),
            )
            eng = eng2[c]
            # heads 0-9 (dil=1): merged scatter into the compact per-chunk
            # strip plane (local m = q + j), then immediate readback
            eng.dma_start(
                out=AP(wplA[c], 0, [[WLA + 1, 128], [SA, 10], [1, KC]]),
                in_=AP(w.tensor, w.offset, [[HJ, 128], [KC, 10], [1, KC]]),
            )
            ta = acts.tile([128, WLA], f16, name=f"wplA{c}")
            H5 = 5 * SA
            eng.dma_start(out=ta[:, 0:H5], in_=AP(wplA[c], 0, [[WLA, 128], [1, H5]]))
            eng.dma_start(out=ta[:, H5:], in_=AP(wplA[c], H5, [[WLA, 128], [1, WLA - H5]]))
            for i, h in enumerate(range(10, HEADS)):
                dil = DIL_H[h]
                eng.dma_start(
                    out=AP(wplB[c], WOFFB[i] + (c * 128) // dil,
                           [[WLDB + 512 // dil, dil],
                            [dil * WLDB + 1, 128 // dil], [1, KC]]),
                    in_=AP(w.tensor, w.offset + h * KC, [[HJ, 128], [1, KC]]),
                )
            tb = acts.tile([128, WLDB], f16, name=f"wplB{c}")
            eng.dma_start(out=tb[:, 0:1024], in_=AP(wplB[c], 0, [[WLDB, 128], [1, 1024]]))
            eng.dma_start(out=tb[:, 1024:], in_=AP(wplB[c], 1024, [[WLDB, 128], [1, 1024]]))
            return ta, tb

        # ---------------- interleave: Sk/softmax chains with V projection
        sk_phase(0, [0, 1])
        vproj_packs([0])
        sk_phase(1, [0, 1])
        vproj_packs([1])
        sk_phase(0, [2, 3, 4])
        wpl0 = softmax_scatter(0)
        vproj_packs([2])
        sk_phase(1, [2, 3, 4])
        wpl1 = softmax_scatter(1)
        vproj_packs([3, 4, 5])
        wpl = [wpl0, wpl1]

        # ---------------- W^T via PE transposes + PV + collapse, one q-chunk
        # stream at a time.
        atiles = {}
        cpy3 = [nc.vector, nc.scalar]
        ncp = 0
        outsb = [work.tile([128, D], f32, name=f"osb{c}", tag="osb", bufs=2)
                 for c in range(2)]

        def out_dma(cc, half):
            a = half * 320
            eng2[cc].dma_start(
                out=AP(out_d, cc * 128 * D + a, [[D, 128], [1, 320]]),
                in_=outsb[cc][:, a : a + 320])

        for c in range(2):
            pcs = {}

            def collapse_c(h, half, start):
                if half not in pcs:
                    pcs[half] = ps_co.tile([128, 320], f32, name=f"pc{c}{half}",
                                           tag="co")
                nc.tensor.matmul(pcs[half], atiles[(h, c)],
                                 CkT[:, h, half * 320 : half * 320 + 320],
                                 start=start, stop=False, skip_group_check=True)

            def collapse_fin(half):
                # fold Cb in as a rank-1 accumulation and close the group
                nc.tensor.matmul(pcs[half], ones1,
                                 CbH[:, half * 320 : half * 320 + 320],
                                 start=False, stop=True, skip_group_check=True)

            def mview_of(h):
                if h < 10:
                    # compact strip: (plane col, vtile mc, row count)
                    return [(h * SA, c, 128), (h * SA + 128, c + 1, KC)]
                return [(WOFFB[h - 10] + mc * 128, mc, 128)
                        for mc in range(MC_H[h]) if c in _live_cs(h, mc)]

            pos = {}

            def emit_batch(batch, src, grp):
                nonlocal ncp
                nb = len(batch)
                ptp = ps_sm.tile([128, nb * 128], f16, name="ptp", tag="sm")
                for i, (coff, mc, rows) in enumerate(batch):
                    # always transpose a full 128-col chunk: rows beyond the
                    # live count land in wt partitions the PV never contracts
                    nc.tensor.transpose(
                        ptp[:, i * 128 : i * 128 + 128],
                        src[:, coff : coff + 128],
                        identh)
                wt = wftp.tile([128, nb * 128], f16, name="wft", tag="wft")
                eng = cpy3[ncp % 2]
                if eng is nc.scalar:
                    eng.copy(wt, ptp)
                else:
                    eng.tensor_copy(wt, ptp)
                ncp += 1
                for i, (coff, mc, rows) in enumerate(batch):
                    pos[(grp, coff)] = (wt, i)

            for h in range(HEADS):
                p, hh = PACK_OF_H[h]
                pat = ps_at.tile([DH, 128], f32, name=f"pat{h}{c}", tag="at")
                mview = mview_of(h)
                src = wpl[c][0] if h < 10 else wpl[c][1]
                if (h < 10, mview[0][0]) not in pos:
                    if h < 10:
                        # pair adjacent dil=1 heads: one 4-slice batch per pair
                        emit_batch(mview_of(h) + mview_of(h + 1), src, True)
                    else:
                        for i in range(0, len(mview), 4):
                            emit_batch(mview[i : i + 4], src, False)
                for i, (coff, mc, rows) in enumerate(mview):
                    wt, j = pos[(h < 10, coff)]
                    nc.tensor.matmul(pat,
                                     vtiles[(p, mc)][0:rows,
                                                     hh * DH : hh * DH + DH],
                                     wt[0:rows, j * 128 : j * 128 + 128],
                                     start=(i == 0), stop=(i == len(mview) - 1),
                                     skip_group_check=True)
                at = actp.tile([DH, 128], f16, name=f"at{h}{c}", tag="at", bufs=28)
                eng = cpy3[(h + c) % 2]
                if eng is nc.scalar:
                    eng.activation(at, pat, mybir.ActivationFunctionType.Identity,
                                   bias=VbT[:, h : h + 1], scale=1.0)
                else:
                    eng.tensor_add(
                        at, pat,
                        AP(bias3.tensor, bias3.offset + 2 * SUBHEADS + h,
                           [[2 * SUBHEADS + HEADS, DH], [0, 128]]))
                atiles[(h, c)] = at
                if h > 0:
                    collapse_c(h - 1, 0, start=(h == 1))
            collapse_c(HEADS - 1, 0, start=False)
            collapse_fin(0)
            nc.vector.tensor_copy(outsb[c][:, 0:320], pcs[0])
            out_dma(c, 0)
            for h in range(HEADS):
                collapse_c(h, 1, start=(h == 0))
            collapse_fin(1)
            nc.scalar.copy(outsb[c][:, 320:640], pcs[1])
            out_dma(c, 1)

    nc.finalize()
    return nc


def _pack_rows(x, nchunk):
    """[nchunk*128, F] -> [128, nchunk*F] partition-major contiguous."""
    F = x.shape[1]
    return np.ascontiguousarray(
        x.reshape(nchunk, DH, F).transpose(1, 0, 2).reshape(DH, nchunk * F))


def _prep_in_maps(inputs):
    h16 = np.float16
    query = np.asarray(inputs["query"], np.float32)
    key = np.asarray(inputs["key"], np.float32)
    value = np.asarray(inputs["value"], np.float32)
    Qk = np.asarray(inputs["Qk"], np.float32)
    Qb = np.asarray(inputs["Qb"], np.float32)
    Kk = np.asarray(inputs["Kk"], np.float32)
    Kb = np.asarray(inputs["Kb"], np.float32)
    Vk = np.asarray(inputs["Vk"], np.float32)
    Vb = np.asarray(inputs["Vb"], np.float32)
    Sk = np.asarray(inputs["Sk"], np.float32)
    Sb = np.asarray(inputs["Sb"], np.float32)
    Ck = np.asarray(inputs["Ck"], np.float32)
    Cb = np.asarray(inputs["Cb"], np.float32)

    def pack_w(Wk):  # [5, 128, 640] -> [128 d2, 25*128]
        WkT = Wk.transpose(0, 2, 1).reshape(SUBHEADS, SUBHEADS, DH, DH)
        return np.ascontiguousarray(
            WkT.transpose(2, 0, 1, 3).reshape(DH, SUBHEADS * SUBHEADS * DH)).astype(h16)

    QkTp = pack_w(Qk)
    KkTp = pack_w(Kk)
    VkT = Vk.transpose(0, 2, 1)                                    # [14, 640, 128]
    VGH = [[0, 1, 2, 3], [4, 5, 6, 7], [8, 9, 10, 11], [12, 13]]
    Vgp = [_pack_rows(np.concatenate([VkT[h] for h in g], axis=1), SUBHEADS).astype(h16)
           for g in VGH]
    SkT = np.ascontiguousarray(Sk.transpose(2, 0, 1).reshape(KC, HJ)).astype(h16)
    SbHr = np.ascontiguousarray(Sb.reshape(1, HJ)).astype(h16)
    CbHr = np.ascontiguousarray(Cb.reshape(1, D)).astype(h16)
    bias3 = np.ascontiguousarray(
        np.concatenate([Qb.T, Kb.T, Vb.T], axis=1))                # [128, 24]
    CkTp = _pack_rows(np.ascontiguousarray(Ck.T), HEADS).astype(h16)

    in_maps = []
    for c in range(NCORES):
        b, t0 = c // 4, (c % 4) * Q
        kpad = np.zeros((KV, D), np.float32)
        vpad = np.zeros((KV, D), np.float32)
        lo, hi = max(0, t0 - HALO), min(N, t0 + Q + 132)
        kpad[lo - (t0 - HALO) : hi - (t0 - HALO)] = key[b, lo:hi]
        vpad[lo - (t0 - HALO) : hi - (t0 - HALO)] = value[b, lo:hi]
        m = {
            "qT": _pack_rows(query[b, t0 : t0 + Q].T, SUBHEADS).astype(h16),
            "kT": _pack_rows(kpad.T, SUBHEADS).astype(h16),
            "vT": _pack_rows(vpad.T, SUBHEADS).astype(h16),
            "QkT": QkTp, "KkT": KkTp,
            "SkT": SkT, "SbH": SbHr, "CbH": CbHr, "bias3": bias3,
            "CkT": CkTp,
        }
        for i in range(4):
            m[f"Vg{i}"] = Vgp[i]
        in_maps.append(m)
    return in_maps


def _run(inputs, trace=False, tmpdir=None):
    global _BUILT
    _inject_ntff_hook()
    from concourse.bass_utils import run_bass_kernel_spmd

    if _BUILT is None:
        _BUILT = _build()
    in_maps = _prep_in_maps(inputs)
    r = run_bass_kernel_spmd(_BUILT, in_maps, core_ids=list(range(NCORES)),
                             trace=trace, tmpdir=tmpdir)
    out = np.empty((B, N, D), np.float32)
    for c in range(NCORES):
        b, t0 = c // 4, (c % 4) * Q
        out[b, t0 : t0 + Q] = r.results[c]["out"]
    return out, r


def kernel(**inputs) -> np.ndarray:
    out, _ = _run(inputs, trace=False)
    return out
